# revision 1
# baseline (speedup 1.0000x reference)
"""GQA (16 query heads, 4 KV groups) forward kernel for 8 Trainium2 NeuronCores.

Sharding: core = (batch b in 0..1) x (kv-group g in 0..3).  Each core owns one
batch element and one whole KV group (4 query heads), computing the output
slice out[b, :, g*512:(g+1)*512].

Per-core plan (all matmul inputs bf16, fp32 PSUM accumulation):
  - x^T loaded straight from DRAM via xbar DMA-transpose (bf16).
  - Q^T [128, T] per head, K^T [128, T], V^T -> V natural [T, 128] on PE.
  - Attention in transposed-score layout: S^T(kb, qc) = K_blk @ Q^T_chunk
    ([128 k x 512 q] blocks), exp on ACT (scale 1/sqrt(hd) folded in), causal
    mask via gpsimd affine_select (zeroing), softmax denominators accumulated
    on DVE + reduced via PE transpose, P@V accumulated in PSUM with V natural
    as the stationary operand, final PE transpose + 1/sum scaling.
"""

import sys

if "/opt/trn_rl_repo" not in sys.path:
    sys.path.insert(0, "/opt/trn_rl_repo")

import ml_dtypes
import numpy as np

import concourse.bass as bass
import concourse.mybir as mybir
import concourse.tile as tile
from concourse import bacc
from concourse.bass_utils import run_bass_kernel_spmd
from concourse.masks import make_identity

B, T, C = 2, 2048, 2048
HEADS, GROUPS = 16, 4
HD = C // HEADS          # 128 head dim
H2G = HEADS // GROUPS    # 4 query heads per group
DG = H2G * HD            # 512 output cols per core
DKV = HD                 # 128 kv dim per group
NCT = C // 128           # 16 contraction tiles
NQC = T // 512           # 4 query chunks
NKB = T // 128           # 16 key blocks
SCALE = HD ** -0.5

F32 = mybir.dt.float32
BF16 = mybir.dt.bfloat16


def _body(tc, xb, wqt, wkt, wvt, out_d):
    nc = tc.nc
    act_exp = mybir.ActivationFunctionType.Exp
    axis_x = mybir.AxisListType.X
    alu_add = mybir.AluOpType.add
    is_ge = mybir.AluOpType.is_ge

    with (
        tc.tile_pool(name="const", bufs=1) as cpool,
        tc.tile_pool(name="data", bufs=1) as data,
    ):
        id_b = cpool.tile([128, 128], BF16)
        make_identity(nc, id_b)
        id_f = cpool.tile([128, 128], F32)
        make_identity(nc, id_f)

        xT = data.tile([128, NCT, T], BF16)    # x^T: [c%128, c//128, t]
        wq = data.tile([128, NCT, DG], BF16)   # Wq^T tiles [c%128, c//128, d]
        wk = data.tile([128, NCT, DKV], BF16)
        wv = data.tile([128, NCT, DKV], BF16)
        kT = data.tile([128, T], BF16)         # K^T: [d, t]
        vn = data.tile([128, NKB, DKV], BF16)  # V natural: [t%128, t//128, d]

        for ci in range(NCT):
            nc.sync.dma_start(out=xT[:, ci, :], in_=xb[ci * 128:(ci + 1) * 128, :])
            nc.sync.dma_start(out=wq[:, ci, :], in_=wqt[ci * 128:(ci + 1) * 128, :])
            nc.sync.dma_start(out=wk[:, ci, :], in_=wkt[ci * 128:(ci + 1) * 128, :])
            nc.sync.dma_start(out=wv[:, ci, :], in_=wvt[ci * 128:(ci + 1) * 128, :])

        # ---- K/V projections ----
        with (
            tc.tile_pool(name="proj_ps", bufs=4, space="PSUM") as pp,
            tc.tile_pool(name="vt_stage", bufs=1) as vstg,
            tc.tile_pool(name="vt_ps", bufs=2, space="PSUM") as vtp,
        ):
            for qc in range(NQC):
                ps = pp.tile([128, 512], F32, tag="proj")
                for ci in range(NCT):
                    nc.tensor.matmul(
                        ps[:], wk[:, ci, :], xT[:, ci, qc * 512:(qc + 1) * 512],
                        start=(ci == 0), stop=(ci == NCT - 1),
                    )
                nc.vector.tensor_copy(kT[:, qc * 512:(qc + 1) * 512], ps[:])
            vT = vstg.tile([128, T], BF16)
            for qc in range(NQC):
                ps = pp.tile([128, 512], F32, tag="proj")
                for ci in range(NCT):
                    nc.tensor.matmul(
                        ps[:], wv[:, ci, :], xT[:, ci, qc * 512:(qc + 1) * 512],
                        start=(ci == 0), stop=(ci == NCT - 1),
                    )
                nc.vector.tensor_copy(vT[:, qc * 512:(qc + 1) * 512], ps[:])
            for tb in range(NKB):
                pt = vtp.tile([128, 128], BF16, tag="vtp")
                nc.tensor.transpose(pt[:], vT[:, tb * 128:(tb + 1) * 128], id_b)
                nc.vector.tensor_copy(vn[:, tb, :], pt[:])

        # ---- attention, with per-chunk Q^T production interleaved ----
        with (
            tc.tile_pool(name="qt_ps", bufs=2, space="PSUM") as qtp,
            tc.tile_pool(name="st_ps", bufs=2, space="PSUM") as stp,
            tc.tile_pool(name="pv_ps", bufs=1, space="PSUM") as pvp,
            tc.tile_pool(name="qt_sb", bufs=2) as qtsb,
            tc.tile_pool(name="ex_sb", bufs=10) as expool,
            tc.tile_pool(name="sum_sb", bufs=2) as sump,
            tc.tile_pool(name="o_sb", bufs=2) as outp,
            tc.tile_pool(name="r_sb", bufs=2) as rsp,
        ):
            def make_qt(qc):
                # Q^T chunk [d, h, 512] for all 4 heads at this q-chunk
                qt = qtsb.tile([128, H2G, 512], BF16, tag="qt", name=f"qt{qc}")
                for h in range(H2G):
                    ps = qtp.tile([128, 512], F32, tag="qtp", name=f"qtp{qc}_{h}")
                    for ci in range(NCT):
                        nc.tensor.matmul(
                            ps[:],
                            wq[:, ci, h * 128:(h + 1) * 128],
                            xT[:, ci, qc * 512:(qc + 1) * 512],
                            start=(ci == 0), stop=(ci == NCT - 1),
                        )
                    nc.vector.tensor_copy(qt[:, h, :], ps[:])
                return qt

            qt_next = make_qt(0)
            for qc in range(NQC):
                qt = qt_next
                pv = [
                    pvp.tile([128, 512], F32, tag=f"pv{h}", name=f"pv{h}")
                    for h in range(H2G)
                ]
                sums = [
                    sump.tile([128, 512], F32, tag=f"sum{h}", name=f"sum{h}")
                    for h in range(H2G)
                ]
                osb = outp.tile([128, 4, DG], F32, tag="osb")
                nkb = 4 * qc + 4
                for kb in range(nkb):
                    diag = kb >= 4 * qc
                    exs = []
                    # all 4 scores matmuls share the stationary K^T block
                    for h in range(H2G):
                        st = stp.tile([128, 512], F32, tag="st", name=f"st{h}")
                        nc.tensor.matmul(
                            st[:],
                            kT[:, kb * 128:(kb + 1) * 128],
                            qt[:, h, :],
                            start=True, stop=True,
                        )
                        ex = expool.tile([128, 512], BF16, tag="ex", name=f"ex{h}")
                        nc.scalar.activation(ex[:], st[:], act_exp, scale=SCALE)
                        if diag:
                            # keep where (qc*512 + f) >= (kb*128 + p)
                            nc.gpsimd.affine_select(
                                out=ex[:], in_=ex[:],
                                compare_op=is_ge,
                                fill=0.0,
                                base=qc * 512 - kb * 128,
                                pattern=[[1, 512]],
                                channel_multiplier=-1,
                            )
                        if kb == 0:
                            nc.vector.tensor_copy(sums[h][:], ex[:])
                        else:
                            nc.vector.tensor_add(sums[h][:], sums[h][:], ex[:])
                        exs.append(ex)
                    # all 4 PV matmuls share the stationary V block
                    for h in range(H2G):
                        nc.tensor.matmul(
                            pv[h][:], vn[:, kb, :], exs[h][:],
                            start=(kb == 0), stop=(kb == nkb - 1),
                        )
                # produce next chunk's Q^T before this chunk's wrap-up so PE
                # stays dense while DVE finishes the softmax denominators
                if qc + 1 < NQC:
                    qt_next = make_qt(qc + 1)
                for h in range(H2G):
                    rsum = rsp.tile([128, 4], F32, tag="rsum")
                    rinv = rsp.tile([128, 4], F32, tag="rinv")
                    for j in range(4):
                        tp = stp.tile([128, 128], F32, tag="st", name=f"tr{j}")
                        nc.tensor.transpose(tp[:], sums[h][:, j * 128:(j + 1) * 128], id_f)
                        nc.vector.tensor_reduce(rsum[:, j:j + 1], tp[:], axis=axis_x, op=alu_add)
                    nc.vector.reciprocal(rinv[:], rsum[:])
                    ot = outp.tile([128, 512], F32, tag="ot")
                    nc.vector.tensor_copy(ot[:], pv[h][:])
                    for j in range(4):
                        tp2 = stp.tile([128, 128], F32, tag="st", name=f"tr2{j}")
                        nc.tensor.transpose(tp2[:], ot[:, j * 128:(j + 1) * 128], id_f)
                        nc.vector.tensor_scalar_mul(
                            osb[:, j, h * 128:(h + 1) * 128], tp2[:], rinv[:, j:j + 1]
                        )
                # One store per q-chunk on SWDGE: single DMA per queue keeps
                # each store at a single sync wait (walrus descriptor limit).
                o_view = out_d[qc * 512:(qc + 1) * 512, :].rearrange(
                    "(j p) d -> p j d", p=128
                )
                nc.gpsimd.dma_start(out=o_view, in_=osb[:, :, :])


def build_nc():
    # Bacc (not raw Bass): its finalize passes split multi-sem waits
    # (move_matmul_waits_to_ldweights / generate_event_semaphores) to meet the
    # 1-wait-per-instruction hardware constraint walrus enforces.
    nc = bacc.Bacc("TRN2", target_bir_lowering=False)
    # xb is x[b] pre-transposed on the host: [C, T] bf16
    xb = nc.declare_dram_parameter("xb", [C, T], BF16, isOutput=False)
    wqt = nc.declare_dram_parameter("wqt", [C, DG], BF16, isOutput=False)
    wkt = nc.declare_dram_parameter("wkt", [C, DKV], BF16, isOutput=False)
    wvt = nc.declare_dram_parameter("wvt", [C, DKV], BF16, isOutput=False)
    out_d = nc.declare_dram_parameter("out", [T, DG], F32, isOutput=True)
    with tile.TileContext(nc) as tc:
        _body(tc, xb, wqt, wkt, wvt, out_d)
    nc.compile()
    return nc


def make_in_maps(x, Wq, Wk, Wv):
    bf = ml_dtypes.bfloat16
    in_maps = []
    for b in range(B):
        xb = np.ascontiguousarray(x[b].T).astype(bf)
        for g in range(GROUPS):
            in_maps.append({
                "xb": xb,
                "wqt": np.ascontiguousarray(Wq[g * DG:(g + 1) * DG].T).astype(bf),
                "wkt": np.ascontiguousarray(Wk[g * DKV:(g + 1) * DKV].T).astype(bf),
                "wvt": np.ascontiguousarray(Wv[g * DKV:(g + 1) * DKV].T).astype(bf),
            })
    return in_maps


def assemble(results):
    out = np.empty((B, T, C), np.float32)
    for i, res in enumerate(results):
        b, g = divmod(i, GROUPS)
        out[b, :, g * DG:(g + 1) * DG] = res["out"]
    return out


def run(x, Wq, Wk, Wv, **spmd_kwargs):
    nc = build_nc()
    in_maps = make_in_maps(x, Wq, Wk, Wv)
    return run_bass_kernel_spmd(nc, in_maps, list(range(8)), **spmd_kwargs)


def kernel(x, Wq, Wk, Wv):
    return assemble(run(x, Wq, Wk, Wv).results)

